# revision 1
# baseline (speedup 1.0000x reference)
"""Bass/Trainium2 kernel for nn_EvoBinarizedLayer.

Reference computation (P=16 populations, B=512, I=O=2048, all values 0/1):
    out[p,b,o] = sum_i x[p,b,i]*w0[p,i,o] + (1-x[p,b,i])*w1[p,i,o]

Strategy:
  - Shard population dim P across 8 cores (2 pops/core), embarrassingly parallel.
  - Cast x/w to fp8e4m3 on host (0/1 values are exact); compute notx = 1-x on
    device (ACT/DVE); accumulate x@w0 + notx@w1 into the same PSUM bank via a
    single K=4096 "concat" contraction -> one accumulation group, no bias pass.
  - fp8 DoubleRow matmuls (K=256 per MM) for 2x PE throughput.
  - PSUM f32 accumulation of 0/1 products is exact (max 4096 < 2^24), so the
    result is bit-exact vs the f32 reference.

Host-side work is layout only: slicing, transpose, dtype cast, and the final
gather. All arithmetic (notx, matmuls) happens on device.
"""

import numpy as np
import ml_dtypes

from concourse import bacc, tile, mybir
from concourse.bass_utils import run_bass_kernel_spmd

P_TOT, B, I, O = 16, 512, 2048, 2048
N_CORES = 8
PPC = P_TOT // N_CORES  # pops per core = 2
PART = 128

FP8 = mybir.dt.float8e4
F32 = mybir.dt.float32
NP_FP8 = ml_dtypes.float8_e4m3


def build_nc(ppc=PPC, b=B, i_dim=I, o_dim=O, n_cores=N_CORES, use_dr=True):
    """Build + compile the per-core Bass program (SPMD: same program, 8 cores)."""
    kt = i_dim // PART          # k-subtiles per weight tensor (16)
    nb = o_dim // 512           # o-blocks (4)
    mb = b // PART              # b-subtiles (4)
    DR = mybir.MatmulPerfMode.DoubleRow if use_dr else None
    kstep = 2 if use_dr else 1

    nc = bacc.Bacc("TRN2", target_bir_lowering=False, debug=False,
                   num_devices=n_cores)

    xt_d = nc.dram_tensor("xt", [ppc, PART, kt, b], FP8, kind="ExternalInput")
    w0_d = nc.dram_tensor("w0", [ppc, nb, PART, kt, 512], FP8, kind="ExternalInput")
    w1_d = nc.dram_tensor("w1", [ppc, nb, PART, kt, 512], FP8, kind="ExternalInput")
    out_d = nc.dram_tensor("out", [ppc, b, o_dim], F32, kind="ExternalOutput")

    with tile.TileContext(nc) as tc:
        with (
            tc.tile_pool(name="xpool", bufs=2) as xpool,
            tc.tile_pool(name="wpool", bufs=4) as wpool,
            tc.tile_pool(name="opool", bufs=4) as opool,
            tc.tile_pool(name="pspool", bufs=4, space="PSUM") as pspool,
        ):
            for pop in range(ppc):
                xt = xpool.tile([PART, kt, b], FP8, tag="xt")
                nxt = xpool.tile([PART, kt, b], FP8, tag="nxt")
                nc.sync.dma_start(out=xt[:], in_=xt_d.ap()[pop])
                # notx = 1 - x  ==  (x * -1) + 1
                nc.vector.tensor_scalar(
                    nxt[:], xt[:], -1.0, 1.0,
                    mybir.AluOpType.mult, mybir.AluOpType.add,
                )
                for nbi in range(nb):
                    w0t = wpool.tile([PART, kt, 512], FP8, tag="w")
                    w1t = wpool.tile([PART, kt, 512], FP8, tag="w")
                    nc.sync.dma_start(out=w0t[:], in_=w0_d.ap()[pop, nbi])
                    nc.sync.dma_start(out=w1t[:], in_=w1_d.ap()[pop, nbi])
                    for m in range(mb):
                        ps = pspool.tile([PART, 512], F32)
                        msl = slice(m * PART, (m + 1) * PART)
                        nk = kt // kstep
                        for kd in range(nk):
                            ksl = slice(kd * kstep, (kd + 1) * kstep)
                            nc.tensor.matmul(
                                ps[:], lhsT=xt[:, ksl, msl], rhs=w0t[:, ksl, :],
                                start=(kd == 0), stop=False, perf_mode=DR,
                            )
                        for kd in range(nk):
                            ksl = slice(kd * kstep, (kd + 1) * kstep)
                            nc.tensor.matmul(
                                ps[:], lhsT=nxt[:, ksl, msl], rhs=w1t[:, ksl, :],
                                start=False, stop=(kd == nk - 1), perf_mode=DR,
                            )
                        ot = opool.tile([PART, 512], F32)
                        nc.vector.tensor_copy(ot[:], ps[:])
                        nc.sync.dma_start(
                            out=out_d.ap()[pop, msl, nbi * 512:(nbi + 1) * 512],
                            in_=ot[:],
                        )
    nc.compile()
    return nc


def prep_core_inputs(x, w, core, ppc=PPC):
    """Layout-only host prep for one core: slice pops, transpose x, tile, cast."""
    p0 = core * ppc
    b, i_dim = x.shape[1], x.shape[2]
    o_dim = w.shape[4]
    kt = i_dim // PART
    nb = o_dim // 512
    xs = x[p0:p0 + ppc]                       # [ppc, B, I]
    # xT partition-tiled: [ppc, 128, kt, B];  xt[p, kp, kti, b] = x[p, b, kti*128+kp]
    xt = np.ascontiguousarray(
        xs.reshape(ppc, b, kt, PART).transpose(0, 3, 2, 1)
    ).astype(NP_FP8)
    ws = w[:, p0:p0 + ppc, 0]                 # [2, ppc, I, O]
    # [2, ppc, nb, 128, kt, 512]; wt[j,p,nbi,kp,kti,no] = w[j,p,kti*128+kp, nbi*512+no]
    wt = np.ascontiguousarray(
        ws.reshape(2, ppc, kt, PART, nb, 512).transpose(0, 1, 4, 3, 2, 5)
    ).astype(NP_FP8)
    return {"xt": xt, "w0": wt[0], "w1": wt[1]}


_NC_CACHE = {}


def _get_nc():
    if "nc" not in _NC_CACHE:
        _NC_CACHE["nc"] = build_nc()
    return _NC_CACHE["nc"]


def kernel(x, w):
    x = np.asarray(x)
    w = np.asarray(w)
    nc = _get_nc()
    in_maps = [prep_core_inputs(x, w, c) for c in range(N_CORES)]
    res = run_bass_kernel_spmd(nc, in_maps, list(range(N_CORES)))
    out = np.concatenate([res.results[c]["out"] for c in range(N_CORES)], axis=0)
    return np.ascontiguousarray(out.astype(np.float32))
